# revision 1
# baseline (speedup 1.0000x reference)
"""Trainium2 Bass kernel for a 3-layer LSTM decoder with Bahdanau attention.

Strategy (8 NeuronCores, data-parallel over time windows):
  The output MLP never feeds back into the recurrence (teacher forcing), so
  the sequential part is only the 3-layer LSTM chain. Each core processes a
  64-step time window (32-step output chunk + 32-step halo) and solves the
  recurrence by Picard fixed-point iteration: all timesteps are updated in
  parallel from the previous iterate, with the linear cell-state recurrence
  c_t = sig(f_t)*c_{t-1} + sig(i_t)*tanh(g_t) solved exactly each iteration
  by the hardware scan instruction. The weights are tiny (sigma=0.05), so the
  map is contractive; K iterations push the truncation error to ~1e-6.

  Attention is evaluated by a 3rd-order Taylor expansion of
  tanh(VOut + att_W h2 + b) around the t-independent base VOut + b: the
  per-step [256,1024] tanh field collapses into 3 small matmuls against
  precomputed derivative fields D1, D2, D3.

Everything on-chip is laid out "H-major": [hidden/gate on partitions, time on
the free dimension], so no transposes are needed in the recurrence.
"""

import numpy as np

H = 256          # hidden
V = 47           # vocab
S = 1024         # encoder frames
TN = 256         # decode steps
G = 4 * H        # gate width 1024
TW = 56          # per-core time window (32 out + 24 halo)
CHUNK = 32       # output chunk per core
NCORES = 8
K_BF = 6         # bf16 Picard iterations
K_FP = 2         # fp32 polish iterations
K_ITERS = K_BF + K_FP

# ---------------------------------------------------------------- blob layout
# One [128, C] fp32 blob per core holding every constant in its exact SBUF
# tile layout. Offsets are column cursors shared by host packer and builder.
_layout = {}
_cursor = 0


def _span(name, cols):
    global _cursor
    _layout[name] = (_cursor, cols)
    _cursor += cols
    return _layout[name]


for _l in (1, 2, 3):
    _span(f"Whh{_l}", 16 * 128)          # W_hh.T chunk grid (k*8+m)*128
    if _l > 1:
        _span(f"Wih{_l}", 16 * 128)
_span("Wih1", 16 * 128)
_span("emb", 256)                        # [47,256] padded to 128 partitions
_span("onehot", TW)                      # [47,TW] padded
_span("bih", 24)                         # 3 layers x [128,8]
_span("bhh", 24)
_span("hinit", 6)                        # 3 layers x [128,2]
_span("cinit", 6)
_span("attWT", 4 * 128)                  # att_W.T grid (2k x 2m)
_span("attVT", 4 * 128)                  # att_V.T grid
_span("attb", 2)                         # [256,1] H-major
_span("av", 2)                           # att_vector H-major
_span("encT", 2 * 1024)                  # enc.T [256,1024] H-major
_span("enc", 16 * 128)                   # enc [1024,256] chunk grid (8k x 2m)
_span("w1T", 8 * 128)                    # mlp_w1.T grid (4k x 2m)
_span("w2T", 4 * 128)                    # mlp_w2.T grid (2k x 2m)
_span("w3T", 2 * V)                      # mlp_w3.T chunks [128,47] x2
_span("b1", 2)
_span("b2", 2)
_span("b3", 1)                           # [47,1] padded
_span("ident", 128)
_span("ones", TW)                        # ones block, row 0 used as [1,TW]
_span("const1", 1)                       # column of 1.0
_span("constm1", 1)                      # column of -1.0
_span("constm13", 1)                     # column of -1/3
BLOB_C = _cursor

# bf16 blob: the six LSTM weight grids, DMA'd first for a fast start
_layout16 = {}
_c16 = 0
for _l in (1, 2, 3):
    for _w in ("hh", "ih"):
        if _l == 1 and _w == "ih":
            continue
        _layout16[f"W{_w}{_l}"] = (_c16, 2048)
        _c16 += 2048
_layout16["ident16"] = (_c16, 128)
_c16 += 128
BLOB16_C = _c16


def _gate_perm():
    # reorder gates i,f,g,o -> i,f,o,g so sigmoid gates are contiguous
    r = np.arange(H)
    return np.concatenate([r, H + r, 3 * H + r, 2 * H + r])


def _grid_wT(W):
    """W [out,in] -> W.T chunk grid [128, (in//128)*(out//128)*128]."""
    WT = np.ascontiguousarray(W.T.astype(np.float32))   # [in, out]
    kin, mout = WT.shape[0] // 128, WT.shape[1] // 128
    g = np.empty((128, kin * mout * 128), np.float32)
    for k in range(kin):
        for m in range(mout):
            g[:, (k * mout + m) * 128:(k * mout + m + 1) * 128] = \
                WT[k * 128:(k + 1) * 128, m * 128:(m + 1) * 128]
    return g


def _grid_wT_thin(W):
    """W [47,256] -> W.T chunks [128, 2*47]."""
    WT = np.ascontiguousarray(W.T.astype(np.float32))   # [256, 47]
    g = np.empty((128, 2 * V), np.float32)
    for k in range(2):
        g[:, k * V:(k + 1) * V] = WT[k * 128:(k + 1) * 128, :]
    return g


def _hmaj(v):
    """flat [n*128] -> [128, n] H-major chunks."""
    n = v.shape[0] // 128
    return np.ascontiguousarray(v.reshape(n, 128).T.astype(np.float32))


def _put(blob, name, arr, rows=None):
    c0, cols = _layout[name]
    r = arr.shape[0] if rows is None else rows
    assert arr.shape[1] <= cols, (name, arr.shape, cols)
    blob[:r, c0:c0 + arr.shape[1]] = arr


def _pack_blob(inp, core):
    blob = np.zeros((128, BLOB_C), np.float32)
    perm = _gate_perm()
    for l in (1, 2, 3):
        _put(blob, f"Whh{l}", _grid_wT(inp[f"W_hh{l}"][perm]))
        if l > 1:
            _put(blob, f"Wih{l}", _grid_wT(inp[f"W_ih{l}"][perm]))
    _put(blob, "Wih1", _grid_wT(inp["W_ih1"][perm]))
    _put(blob, "emb", inp["emb"].astype(np.float32))            # [47,256]

    lo = 0 if core == 0 else 32 * core - (TW - 32)
    Y = np.asarray(inp["Y"]).astype(np.int64)[lo:lo + TW]
    oh = np.zeros((V, TW), np.float32)
    oh[Y, np.arange(TW)] = 1.0
    _put(blob, "onehot", oh)

    bih = np.concatenate([_hmaj(inp[f"b_ih{l}"][perm]) for l in (1, 2, 3)], 1)
    bhh = np.concatenate([_hmaj(inp[f"b_hh{l}"][perm]) for l in (1, 2, 3)], 1)
    _put(blob, "bih", bih)
    _put(blob, "bhh", bhh)
    if core == 0:
        hi = np.concatenate([_hmaj(np.asarray(inp["h"])[l, 0]) for l in range(3)], 1)
        ci = np.concatenate([_hmaj(np.asarray(inp["c"])[l, 0]) for l in range(3)], 1)
        _put(blob, "hinit", hi)
        _put(blob, "cinit", ci)
    _put(blob, "attWT", _grid_wT(inp["att_W"]))
    _put(blob, "attVT", _grid_wT(inp["att_V"]))
    _put(blob, "attb", _hmaj(inp["att_b"][:, 0]))
    _put(blob, "av", _hmaj(inp["att_vector"][0]))
    enc = inp["outEncoder"].astype(np.float32)                  # [1024,256]
    encT = np.ascontiguousarray(enc.T)                          # [256,1024]
    eT = np.empty((128, 2048), np.float32)
    for k in range(2):
        eT[:, k * 1024:(k + 1) * 1024] = encT[k * 128:(k + 1) * 128, :]
    _put(blob, "encT", eT)
    eg = np.empty((128, 16 * 128), np.float32)
    for k in range(8):
        for m in range(2):
            eg[:, (k * 2 + m) * 128:(k * 2 + m + 1) * 128] = \
                enc[k * 128:(k + 1) * 128, m * 128:(m + 1) * 128]
    _put(blob, "enc", eg)
    _put(blob, "w1T", _grid_wT(inp["mlp_w1"]))
    _put(blob, "w2T", _grid_wT(inp["mlp_w2"]))
    _put(blob, "w3T", _grid_wT_thin(inp["mlp_w3"]))
    _put(blob, "b1", _hmaj(inp["mlp_b1"]))
    _put(blob, "b2", _hmaj(inp["mlp_b2"]))
    _put(blob, "b3", inp["mlp_b3"].astype(np.float32)[:, None])
    _put(blob, "ident", np.eye(128, dtype=np.float32))
    _put(blob, "ones", np.ones((128, TW), np.float32))
    _put(blob, "const1", np.ones((128, 1), np.float32))
    _put(blob, "constm1", -np.ones((128, 1), np.float32))
    _put(blob, "constm13", np.full((128, 1), -1.0 / 3.0, np.float32))

    import ml_dtypes
    blob16 = np.zeros((128, BLOB16_C), ml_dtypes.bfloat16)
    for l in (1, 2, 3):
        c0, cols = _layout16[f"Whh{l}"]
        blob16[:, c0:c0 + cols] = _grid_wT(inp[f"W_hh{l}"][perm])
        if l > 1:
            c0, cols = _layout16[f"Wih{l}"]
            blob16[:, c0:c0 + cols] = _grid_wT(inp[f"W_ih{l}"][perm])
    c0, cols = _layout16["ident16"]
    blob16[:, c0:c0 + cols] = np.eye(128, dtype=np.float32)
    return blob, blob16


# ------------------------------------------------------------------- builder
_NC_CACHE = [None]


def _build():
    import concourse.bacc as bacc
    import concourse.mybir as mybir
    from concourse import tile

    F32 = mybir.dt.float32
    BF16 = mybir.dt.bfloat16
    AF = mybir.ActivationFunctionType
    OP = mybir.AluOpType

    nc = bacc.Bacc("TRN2", target_bir_lowering=False, debug=False,
                   num_devices=NCORES)
    blob_d = nc.dram_tensor("blob", [128, BLOB_C], F32, kind="ExternalInput").ap()
    blob16_d = nc.dram_tensor("blob16", [128, BLOB16_C], BF16,
                              kind="ExternalInput").ap()
    out_d = nc.dram_tensor("out", [V, TW], F32, kind="ExternalOutput").ap()

    with tile.TileContext(nc) as tc:
        import contextlib
        ctx = contextlib.ExitStack()
        with ctx:
            cp = ctx.enter_context(tc.tile_pool(name="consts", bufs=1))
            wp = ctx.enter_context(tc.tile_pool(name="work", bufs=1))
            ewp = ctx.enter_context(tc.tile_pool(name="ew", bufs=3))
            pg = ctx.enter_context(tc.tile_pool(name="pgates", bufs=3,
                                                space="PSUM"))
            pm = ctx.enter_context(tc.tile_pool(name="pmisc", bufs=1,
                                                space="PSUM"))

            # --- constant tiles, one DMA each (ordered by first use)
            def cload(name):
                c0, cols = _layout[name]
                t = cp.tile([128, cols], F32, name=name, tag=name)
                nc.sync.dma_start(t[:], blob_d[:, c0:c0 + cols])
                return t

            def cload16(name):
                c0, cols = _layout16[name]
                t = cp.tile([128, cols], BF16, name=name + "_16",
                            tag=name + "_16")
                nc.sync.dma_start(t[:], blob16_d[:, c0:c0 + cols])
                return t

            # DMA order = first-use order: tiny setup tensors, Wih1 (XW1),
            # ident16, then the bf16 grids, then everything phase-2/fp32.
            emb = cload("emb")
            onehot = cload("onehot")
            bih = cload("bih")
            bhh = cload("bhh")
            hinit = cload("hinit")
            cinit = cload("cinit")
            ones = cload("ones")
            const1 = cload("const1")
            constm13 = cload("constm13")
            wih1 = cload("Wih1")
            ident16 = cload16("ident16")
            g16 = {}
            for _l in (1, 2, 3):
                g16[f"hh{_l}"] = cload16(f"Whh{_l}")
                if _l > 1:
                    g16[f"ih{_l}"] = cload16(f"Wih{_l}")
            grids16 = {1: {"hh": g16["hh1"], "ih": None},
                       2: {"hh": g16["hh2"], "ih": g16["ih2"]},
                       3: {"hh": g16["hh3"], "ih": g16["ih3"]}}
            attVT = cload("attVT")
            attb = cload("attb")
            av = cload("av")
            encT = cload("encT")
            ident = cload("ident")
            whh1 = cload("Whh1")
            wih2 = cload("Wih2")
            whh2 = cload("Whh2")
            wih3 = cload("Wih3")
            whh3 = cload("Whh3")
            attWT = cload("attWT")
            encg = cload("enc")
            w1T = cload("w1T")
            w2T = cload("w2T")
            w3T = cload("w3T")
            b1 = cload("b1")
            b2 = cload("b2")
            b3 = cload("b3")

            grids = {1: {"hh": whh1, "ih": wih1},
                     2: {"hh": whh2, "ih": wih2},
                     3: {"hh": whh3, "ih": wih3}}

            def gchunk(gr, k, m, mout=8):
                i = k * mout + m
                return gr[:, i * 128:(i + 1) * 128]

            # --- combined biases per layer, H-major [128,8]
            bsum = wp.tile([128, 24], F32, tag="bsum")
            nc.vector.tensor_add(bsum[:], bih[:], bhh[:])

            # --- X.T = emb.T @ onehot  -> [128, 2, TW]
            x_ps = pm.tile([128, 2 * TW], F32, tag="pm")
            for m in range(2):
                nc.tensor.matmul(x_ps[:, m * TW:(m + 1) * TW],
                                 emb[:V, m * 128:(m + 1) * 128],
                                 onehot[:V, :], start=True, stop=True)
            x_sb = wp.tile([128, 2 * TW], F32, tag="xsb")
            nc.vector.tensor_copy(x_sb[:], x_ps[:])

            # --- XW1 = W_ih1.T-grid @ X (+ b1sum), H-major [128, 8*TW]
            xw_ps = pg.tile([128, 8 * TW], F32, tag="gates")
            for m in range(8):
                for k in range(2):
                    nc.tensor.matmul(
                        xw_ps[:, m * TW:(m + 1) * TW],
                        gchunk(wih1, k, m),
                        x_sb[:, k * TW:(k + 1) * TW],
                        start=(k == 0), stop=(k == 1))
            xw1 = wp.tile([128, 8 * TW], F32, tag="xw1")
            for m in range(8):
                nc.scalar.activation(xw1[:, m * TW:(m + 1) * TW],
                                     xw_ps[:, m * TW:(m + 1) * TW],
                                     AF.Identity, bias=bsum[:, 0 + m:1 + m])

            # --- attention precompute: VOut, tb, D1, D2, D3, e0 (emitted
            # between the bf16 and fp32 iteration diagonals to fill stalls)
            tb = wp.tile([128, 2 * 1024], F32, tag="tb")
            t2 = wp.tile([128, 2 * 1024], F32, tag="t2")
            d1 = wp.tile([128, 2 * 1024], BF16, tag="d1")
            d2 = wp.tile([128, 2 * 1024], BF16, tag="d2")
            d3 = wp.tile([128, 2 * 1024], BF16, tag="d3")
            e0 = wp.tile([1, 1024], F32, tag="e0")

            def emit_att_precompute():
                vout_ps = pm.tile([128, 512], F32, name="vout_ps", tag="pm")
                for m in range(2):          # h' chunk
                    for h in range(2):      # s half
                        for k in range(2):  # contraction chunk
                            nc.tensor.matmul(
                                vout_ps[:],
                                gchunk(attVT, k, m, mout=2),
                                encT[:, k * 1024 + h * 512:
                                     k * 1024 + (h + 1) * 512],
                                start=(k == 0), stop=(k == 1))
                        nc.scalar.activation(
                            tb[:, m * 1024 + h * 512: m * 1024 + (h + 1) * 512],
                            vout_ps[:], AF.Tanh, bias=attb[:, m:m + 1])
                for q in range(4):
                    sq = slice(q * 512, (q + 1) * 512)
                    nc.vector.tensor_mul(t2[:, sq], tb[:, sq], tb[:, sq])
                for m in range(2):
                    sl = slice(m * 1024, (m + 1) * 1024)
                    nc.scalar.activation(d1[:, sl], t2[:, sl], AF.Identity,
                                         bias=const1[:, 0:1], scale=-1.0)
                # d2 = -tb*(1-tb^2), d3 = (1-tb^2)*(tb^2 - 1/3): the Taylor
                # term signs/scales live here, off the phase-2 critical tail
                for q in range(4):
                    sq = slice(q * 512, (q + 1) * 512)
                    nc.vector.tensor_mul(d2[:, sq], tb[:, sq], d1[:, sq])
                    nc.vector.tensor_scalar_mul(d2[:, sq], d2[:, sq], -1.0)
                for m in range(2):
                    sl = slice(m * 1024, (m + 1) * 1024)
                    nc.scalar.activation(d3[:, sl], t2[:, sl], AF.Identity,
                                         bias=constm13[:, 0:1], scale=1.0)
                for q in range(4):
                    sq = slice(q * 512, (q + 1) * 512)
                    nc.vector.tensor_mul(d3[:, sq], d1[:, sq], d3[:, sq])
                e0_ps = pm.tile([1, 1024], F32, name="e0_ps", tag="pm")
                for h in range(2):
                    for k in range(2):
                        nc.tensor.matmul(
                            e0_ps[:, h * 512:(h + 1) * 512], av[:, k:k + 1],
                            tb[:, k * 1024 + h * 512:k * 1024 + (h + 1) * 512],
                            start=(k == 0), stop=(k == 1))
                nc.vector.tensor_copy(e0[:], e0_ps[:])

            # --- h ping-pong buffers [128, 2*(TW+1)]; col 0 of each chunk=init
            CW = TW + 1
            hbufs = [[wp.tile([128, 2 * CW], BF16, name=f"hb{l}{p}",
                              tag=f"hb{l}{p}")
                      for l in range(3)] for p in range(2)]
            hbufs32 = [[wp.tile([128, 2 * CW], F32, name=f"hf{l}{p}",
                                tag=f"hf{l}{p}")
                        for l in range(3)] for p in range(2)]
            for bufs in (hbufs, hbufs32):
                for p in range(2):
                    for l in range(3):
                        # zero: iteration 0 reads the t-columns as the Picard
                        # zero-init guess, so they must not be garbage
                        nc.gpsimd.memset(bufs[p][l][:], 0.0)
                        dst = bufs[p][l][:].rearrange("p (c u) -> p c u", c=2)
                        nc.vector.tensor_copy(dst[:, :, 0:1],
                                              hinit[:, 2 * l:2 * l + 2]
                                              .rearrange("p (c u) -> p c u", c=2))

            # per-layer additive term: L1 uses XW1 (incl. bias); L2/L3 use the
            # bias broadcast along t, pre-materialized once. Folded into the
            # gate PSUM accumulation via an identity matmul so the elementwise
            # chain reads PSUM directly. bf16 copies serve the bf16 units.
            xadd = [xw1]
            for l in (1, 2):
                bt = wp.tile([128, 8 * TW], F32, name=f"btile{l}",
                             tag=f"btile{l}")
                for m in range(8):
                    nc.vector.tensor_scalar_mul(
                        bt[:, m * TW:(m + 1) * TW], ones[:, 0:TW],
                        bsum[:, 8 * l + m:8 * l + m + 1])
                xadd.append(bt)
            xadd16 = []
            for l in range(3):
                x16 = wp.tile([128, 8 * TW], BF16, name=f"xadd16_{l}",
                              tag=f"xadd16_{l}")
                nc.vector.tensor_copy(x16[:], xadd[l][:])
                xadd16.append(x16)

            # ---------------- Picard iterations (wavefront order) ----------
            def emit_unit(l, it):
                bf = it < K_BF
                hb = hbufs if bf else hbufs32
                gr_set = grids16 if bf else grids
                rb, wb = hb[it % 2], hb[(it + 1) % 2]
                ps = pg.tile([128, 8 * TW], F32, name="ps", tag="gates")
                srcs = [(gr_set[l + 1]["hh"], rb[l], 0)]
                if l > 0:
                    srcs.append((gr_set[l + 1]["ih"], wb[l - 1], 1))
                n_acc = 2 * len(srcs)
                xi, xa = (ident16, xadd16[l]) if bf else (ident, xadd[l])
                for m in range(8):
                    a = 0
                    for gr, src, off in srcs:
                        for k in range(2):
                            nc.tensor.matmul(
                                ps[:, m * TW:(m + 1) * TW],
                                gchunk(gr, k, m),
                                src[:, k * CW + off:k * CW + off + TW],
                                start=(a == 0), stop=False)
                            a += 1
                    nc.tensor.matmul(
                        ps[:, m * TW:(m + 1) * TW], xi[:],
                        xa[:, m * TW:(m + 1) * TW],
                        start=False, stop=True)
                sig = ewp.tile([128, 6 * TW], F32, name="sig", tag="sig")
                tg = ewp.tile([128, 2 * TW], F32, name="tg", tag="tg")
                nc.scalar.activation(sig[:], ps[:, 0:6 * TW], AF.Sigmoid)
                nc.scalar.activation(tg[:], ps[:, 6 * TW:8 * TW], AF.Tanh)
                z = ewp.tile([128, 2 * TW], F32, name="z", tag="z")
                nc.vector.tensor_mul(z[:], sig[:, 0:2 * TW], tg[:])
                cs = ewp.tile([128, 2 * TW], F32, name="cs", tag="cs")
                for j in range(2):
                    nc.vector.tensor_tensor_scan(
                        cs[:, j * TW:(j + 1) * TW],
                        sig[:, 2 * TW + j * TW:2 * TW + (j + 1) * TW],
                        z[:, j * TW:(j + 1) * TW],
                        cinit[:, 2 * l + j:2 * l + j + 1],
                        OP.mult, OP.add)
                tcs = ewp.tile([128, 2 * TW], F32, name="tcs", tag="tcs")
                nc.scalar.activation(tcs[:], cs[:], AF.Tanh)
                dst = wb[l][:].rearrange("p (c u) -> p c u", c=2)[:, :, 1:CW]
                nc.vector.tensor_mul(
                    dst,
                    sig[:, 4 * TW:6 * TW].rearrange("p (c u) -> p c u", c=2),
                    tcs[:].rearrange("p (c u) -> p c u", c=2))
                if it == K_BF - 1:
                    # seed the fp32 buffers for the polish iterations
                    d32 = hbufs32[(it + 1) % 2][l][:] \
                        .rearrange("p (c u) -> p c u", c=2)[:, :, 1:CW]
                    nc.vector.tensor_copy(d32, dst)
                return cs

            # diagonal t = 2*it + l: U(l,it) depends on U(l-1,it) [t-1] and
            # U(l,it-1) [t-2], so emitting by increasing t lets the PE run
            # layer (l, it) while (l+1.., it-1..) elementwise chains drain.
            last_cs = [None]
            for t in range(2 * K_ITERS + 3):
                if t == 2 * K_BF + 1:
                    emit_att_precompute()
                for l in range(3):
                    it = (t - l) // 2
                    if (t - l) % 2 == 0 and 0 <= it < K_ITERS:
                        last_cs[0] = emit_unit(l, it)

            # prefetch the exp activation table: a dummy exp data-dependent on
            # the last unit's cell state runs right as phase 1 drains, hiding
            # the ~2.7us table swap from the phase-2 critical tail.
            dummy = wp.tile([1, 1], F32, tag="dummy")
            nc.scalar.activation(dummy[:], last_cs[0][0:1, 0:1], AF.Exp)

            h2f = hbufs32[K_ITERS % 2][2]
            h2c = [h2f[:, k * CW + 1:k * CW + 1 + TW] for k in range(2)]

            # ---------------- phase 2: attention + MLP ----------------
            ws_ps = pm.tile([128, 2, TW], F32, tag="pm")
            for m in range(2):
                for k in range(2):
                    nc.tensor.matmul(ws_ps[:, m, :],
                                     gchunk(attWT, k, m, mout=2), h2c[k],
                                     start=(k == 0), stop=(k == 1))
            u1 = wp.tile([128, 2 * TW], BF16, tag="u1")
            u2 = wp.tile([128, 2 * TW], BF16, tag="u2")
            u3 = wp.tile([128, 2 * TW], BF16, tag="u3")
            for m in range(2):
                nc.vector.tensor_scalar_mul(u1[:, m * TW:(m + 1) * TW],
                                            ws_ps[:, m, :], av[:, m:m + 1])
            ws_flat = ws_ps[:].rearrange("p c u -> p (c u)")
            nc.vector.tensor_mul(u2[:], u1[:], ws_flat)
            nc.vector.tensor_mul(u3[:], u2[:], ws_flat)

            e_ps = pm.tile([TW, 1024], F32, tag="pm")
            for h in range(2):
                sl = slice(h * 512, (h + 1) * 512)
                nc.tensor.matmul(e_ps[:, sl], ones[0:1, 0:TW], e0[:, sl],
                                 start=True, stop=False)
                for u, d in ((u1, d1), (u2, d2), (u3, d3)):
                    for k in range(2):
                        nc.tensor.matmul(
                            e_ps[:, sl], u[:, k * TW:(k + 1) * TW],
                            d[:, k * 1024 + h * 512:k * 1024 + (h + 1) * 512],
                            start=False, stop=(u is u3 and k == 1))

            # softmax over s (|e| < 0.2, no max-subtraction needed)
            alpha = wp.tile([TW, 1024], F32, tag="alpha")
            asum = wp.tile([TW, 1], F32, tag="asum")
            nc.scalar.activation(alpha[:], e_ps[:], AF.Exp, accum_out=asum[:])
            rsum = wp.tile([TW, 1], F32, tag="rsum")
            nc.vector.reciprocal(rsum[:], asum[:])
            nc.vector.tensor_scalar_mul(alpha[:], alpha[:], rsum[:])

            # transpose alpha -> [1024(s), TW] via PE, then ctx.T = enc.T@a.T
            at_ps = pm.tile([128, 8 * TW], F32, tag="pm")
            for j in range(8):
                nc.tensor.transpose(at_ps[:, j * TW:(j + 1) * TW],
                                    alpha[:, j * 128:(j + 1) * 128],
                                    ident[0:TW, 0:TW])
            at_sb = wp.tile([128, 8 * TW], F32, tag="atsb")
            nc.vector.tensor_copy(at_sb[:], at_ps[:])
            ctx_ps = pm.tile([128, 2, TW], F32, tag="pm")
            for m in range(2):
                for k in range(8):
                    nc.tensor.matmul(ctx_ps[:, m, :],
                                     gchunk(encg, k, m, mout=2),
                                     at_sb[:, k * TW:(k + 1) * TW],
                                     start=(k == 0), stop=(k == 7))
            ctx_sb = wp.tile([128, 2 * TW], F32, tag="ctxsb")
            nc.vector.tensor_copy(ctx_sb[:],
                                  ctx_ps[:].rearrange("p c u -> p (c u)"))

            # MLP: v = [h2; ctx]
            v1_ps = pm.tile([128, 2, TW], F32, tag="pm")
            for m in range(2):
                for k in range(4):
                    rhs = h2c[k] if k < 2 else ctx_sb[:, (k - 2) * TW:(k - 1) * TW]
                    nc.tensor.matmul(v1_ps[:, m, :], gchunk(w1T, k, m, mout=2),
                                     rhs, start=(k == 0), stop=(k == 3))
            v1 = wp.tile([128, 2 * TW], F32, tag="v1")
            for m in range(2):
                nc.scalar.activation(v1[:, m * TW:(m + 1) * TW], v1_ps[:, m, :],
                                     AF.Relu, bias=b1[:, m:m + 1])
            v2_ps = pm.tile([128, 2, TW], F32, tag="pm")
            for m in range(2):
                for k in range(2):
                    nc.tensor.matmul(v2_ps[:, m, :], gchunk(w2T, k, m, mout=2),
                                     v1[:, k * TW:(k + 1) * TW],
                                     start=(k == 0), stop=(k == 1))
            v2 = wp.tile([128, 2 * TW], F32, tag="v2")
            for m in range(2):
                nc.scalar.activation(v2[:, m * TW:(m + 1) * TW], v2_ps[:, m, :],
                                     AF.Relu, bias=b2[:, m:m + 1])
            o_ps = pm.tile([V, TW], F32, tag="pm")
            for k in range(2):
                nc.tensor.matmul(o_ps[:], w3T[:, k * V:(k + 1) * V],
                                 v2[:, k * TW:(k + 1) * TW],
                                 start=(k == 0), stop=(k == 1))
            o_sb = wp.tile([V, TW], F32, tag="osb")
            nc.scalar.activation(o_sb[:], o_ps[:], AF.Identity,
                                 bias=b3[:V, 0:1])
            nc.sync.dma_start(out_d[:], o_sb[:])

    nc.compile()
    return nc


def _run(inp, trace=False):
    if _NC_CACHE[0] is None:
        _NC_CACHE[0] = _build()
    nc = _NC_CACHE[0]
    from concourse.bass_utils import run_bass_kernel_spmd
    in_maps = []
    for k in range(NCORES):
        b32, b16 = _pack_blob(inp, k)
        in_maps.append({"blob": b32, "blob16": b16})
    res = run_bass_kernel_spmd(nc, in_maps, list(range(NCORES)), trace=trace)
    out = np.zeros((TN, 1, V), np.float32)
    for k in range(NCORES):
        o = res.results[k]["out"]          # [47, TW]
        c0 = 0 if k == 0 else TW - 32
        out[32 * k:32 * k + 32, 0, :] = o[:, c0:c0 + 32].T
    return out, res


def kernel(**inputs) -> np.ndarray:
    inp = {k: np.asarray(v) if not np.isscalar(v) else v
           for k, v in inputs.items()}
    out, _ = _run(inp, trace=False)
    return out



# revision 10
# speedup vs baseline: 1.7015x; 1.7015x over previous
"""Trainium2 Bass kernel for a 3-layer LSTM decoder with Bahdanau attention.

Strategy (8 NeuronCores, data-parallel over time windows):
  Each core processes a 48-step time window (32-step output chunk + 16-step
  halo) and solves the teacher-forced recurrence by Picard fixed-point
  iteration (Gauss-Seidel over layers): K bf16 sweeps + one extra layer-3
  polish, with the linear cell recurrence solved exactly per sweep by the
  hardware scan. Weights are tiny (sigma=0.05) so the map is strongly
  contractive.

  Attention uses a 1st-order Taylor expansion of tanh(VOut + att_W h2 + b)
  around the t-independent base: the loop-invariant term e0 = av @ tanh(base)
  is folded into the e-matmul via a broadcast-av stationary, so per-window
  attention is just two small matmul groups against precomputed bf16 fields.

  Everything is bf16 on the PE (1 cycle/row); PSUM accumulation is fp32.
  The three activation-table loads (sigmoid/tanh/exp) are pulled to t=0 by
  dummy activations so they hide under the weight DMA.
"""

import numpy as np

H = 256
V = 47
S = 1024
TN = 256
TW = 48          # per-core window (32 out + 16 halo)
CW = TW + 1      # window + init column
CHUNK = 32
NCORES = 8
K_IT = 5         # bf16 Picard sweeps (gauss-seidel over layers)
# one extra layer-3 polish unit after the K sweeps (always on)

# ---------------------------------------------------------------- blob layout
_layout16 = {}
_c16 = 0


def _span16(name, cols):
    global _c16
    _layout16[name] = (_c16, cols)
    _c16 += cols


_span16("onehot", TW)
_span16("emb", 256)
_span16("ones", TW)
_span16("ident", 128)
_span16("Wih1", 2048)
_span16("Whh1", 2048)
_span16("Wih2", 2048)
_span16("Whh2", 2048)
_span16("Wih3", 2048)
_span16("Whh3", 2048)
_span16("attVT", 512)
_span16("encT", 2048)
_span16("attWT", 512)
_span16("enc", 2048)
_span16("w1T", 1024)
_span16("w2T", 512)
_span16("w3T", 2 * V)
BLOB16_C = _c16

ROW_C = 3 * 1024          # combined biases, one row: layer l at l*1024

_layout32 = {}
_c32 = 0


def _span32(name, cols):
    global _c32
    _layout32[name] = (_c32, cols)
    _c32 += cols


_span32("hinit", 6)
_span32("cinit", 6)
_span32("attb", 2)
_span32("av", 2)
_span32("b1", 2)
_span32("b2", 2)
_span32("b3", 1)
BLOB32_C = _c32


def _gate_perm():
    # reorder gates i,f,g,o -> i,f,o,g so sigmoid gates are contiguous
    r = np.arange(H)
    return np.concatenate([r, H + r, 3 * H + r, 2 * H + r])


def _grid_wT(W):
    """W [out,in] -> W.T chunk grid [128, (in//128)*(out//128)*128]."""
    WT = np.ascontiguousarray(W.T.astype(np.float32))
    kin, mout = WT.shape[0] // 128, WT.shape[1] // 128
    g = np.empty((128, kin * mout * 128), np.float32)
    for k in range(kin):
        for m in range(mout):
            g[:, (k * mout + m) * 128:(k * mout + m + 1) * 128] = \
                WT[k * 128:(k + 1) * 128, m * 128:(m + 1) * 128]
    return g


def _hmaj(v):
    n = v.shape[0] // 128
    return np.ascontiguousarray(v.reshape(n, 128).T.astype(np.float32))


def _pack(inp, core):
    import ml_dtypes
    BF = ml_dtypes.bfloat16
    b16 = np.zeros((128, BLOB16_C), BF)
    row = np.zeros((1, ROW_C), BF)
    b32 = np.zeros((128, BLOB32_C), np.float32)
    perm = _gate_perm()

    def put16(name, arr):
        c0, cols = _layout16[name]
        b16[:arr.shape[0], c0:c0 + arr.shape[1]] = arr.astype(BF)

    def put32(name, arr):
        c0, cols = _layout32[name]
        b32[:arr.shape[0], c0:c0 + arr.shape[1]] = arr

    lo = 0 if core == 0 else CHUNK * core - (TW - CHUNK)
    Y = np.asarray(inp["Y"]).astype(np.int64)[lo:lo + TW]
    oh = np.zeros((V, TW), np.float32)
    oh[Y, np.arange(TW)] = 1.0
    put16("onehot", oh)
    put16("emb", np.asarray(inp["emb"]).astype(np.float32))
    put16("ones", np.ones((128, TW), np.float32))
    put16("ident", np.eye(128, dtype=np.float32))
    for l in (1, 2, 3):
        put16(f"Wih{l}", _grid_wT(np.asarray(inp[f"W_ih{l}"])[perm]))
        put16(f"Whh{l}", _grid_wT(np.asarray(inp[f"W_hh{l}"])[perm]))
        bsum = (np.asarray(inp[f"b_ih{l}"]) + np.asarray(inp[f"b_hh{l}"]))[perm]
        row[0, (l - 1) * 1024:l * 1024] = bsum.astype(np.float32)
    put16("attVT", _grid_wT(np.asarray(inp["att_V"])))
    put16("attWT", _grid_wT(np.asarray(inp["att_W"])))
    enc = np.asarray(inp["outEncoder"]).astype(np.float32)
    encT = np.ascontiguousarray(enc.T)
    eT = np.empty((128, 2048), np.float32)
    for k in range(2):
        eT[:, k * 1024:(k + 1) * 1024] = encT[k * 128:(k + 1) * 128, :]
    put16("encT", eT)
    eg = np.empty((128, 16 * 128), np.float32)
    for k in range(8):
        for m in range(2):
            eg[:, (k * 2 + m) * 128:(k * 2 + m + 1) * 128] = \
                enc[k * 128:(k + 1) * 128, m * 128:(m + 1) * 128]
    put16("enc", eg)
    put16("w1T", _grid_wT(np.asarray(inp["mlp_w1"])))
    put16("w2T", _grid_wT(np.asarray(inp["mlp_w2"])))
    w3T = np.ascontiguousarray(np.asarray(inp["mlp_w3"]).astype(np.float32).T)
    g3 = np.empty((128, 2 * V), np.float32)
    for k in range(2):
        g3[:, k * V:(k + 1) * V] = w3T[k * 128:(k + 1) * 128, :]
    put16("w3T", g3)

    if core == 0:
        hi = np.concatenate([_hmaj(np.asarray(inp["h"])[l, 0]) for l in range(3)], 1)
        ci = np.concatenate([_hmaj(np.asarray(inp["c"])[l, 0]) for l in range(3)], 1)
        put32("hinit", hi)
        put32("cinit", ci)
    put32("attb", _hmaj(np.asarray(inp["att_b"])[:, 0]))
    put32("av", _hmaj(np.asarray(inp["att_vector"])[0]))
    put32("b1", _hmaj(np.asarray(inp["mlp_b1"])))
    put32("b2", _hmaj(np.asarray(inp["mlp_b2"])))
    put32("b3", np.asarray(inp["mlp_b3"]).astype(np.float32)[:, None])
    return b16, row, b32


# ------------------------------------------------------------------- builder
_NC_CACHE = [None]


def _build():
    import concourse.bacc as bacc
    import concourse.mybir as mybir
    from concourse import tile

    F32 = mybir.dt.float32
    BF16 = mybir.dt.bfloat16
    AF = mybir.ActivationFunctionType
    OP = mybir.AluOpType

    nc = bacc.Bacc("TRN2", target_bir_lowering=False, debug=False,
                   num_devices=NCORES)
    b16_d = nc.dram_tensor("blob16", [128, BLOB16_C], BF16,
                           kind="ExternalInput").ap()
    row_d = nc.dram_tensor("brow", [1, ROW_C], BF16, kind="ExternalInput").ap()
    b32_d = nc.dram_tensor("blob32", [128, BLOB32_C], F32,
                           kind="ExternalInput").ap()
    out_d = nc.dram_tensor("out", [V, TW], F32, kind="ExternalOutput").ap()

    with tile.TileContext(nc) as tc:
        import contextlib
        ctx = contextlib.ExitStack()
        with ctx:
            cp = ctx.enter_context(tc.tile_pool(name="consts", bufs=1))
            wp = ctx.enter_context(tc.tile_pool(name="work", bufs=1))
            ewp = ctx.enter_context(tc.tile_pool(name="ew", bufs=3))
            pg = ctx.enter_context(tc.tile_pool(name="pgates", bufs=4,
                                                space="PSUM"))
            pe_pool = ctx.enter_context(tc.tile_pool(name="peps", bufs=1,
                                                     space="PSUM"))
            pm = ctx.enter_context(tc.tile_pool(name="pmisc", bufs=2,
                                                space="PSUM"))

            # --- act-table warmup: pull the 3 table loads to t=0
            warm = wp.tile([1, 4], F32, tag="warm")
            nc.gpsimd.memset(warm[:], 0.0)
            nc.scalar.activation(warm[:, 1:2], warm[:, 0:1], AF.Sigmoid)
            nc.scalar.activation(warm[:, 2:3], warm[:, 0:1], AF.Tanh)
            nc.scalar.activation(warm[:, 3:4], warm[:, 0:1], AF.Exp)

            # --- constant loads (DMA order = first-use order)
            def cload(name):
                c0, cols = _layout16[name]
                t = cp.tile([128, cols], BF16, name=name, tag=name)
                nc.sync.dma_start(t[:], b16_d[:, c0:c0 + cols])
                return t

            onehot = cload("onehot")
            emb = cload("emb")
            brow = cp.tile([1, ROW_C], BF16, name="brow", tag="brow")
            nc.sync.dma_start(brow[:], row_d[:])
            b32t = cp.tile([128, BLOB32_C], F32, name="b32", tag="b32")
            nc.sync.dma_start(b32t[:], b32_d[:])

            def c32(name):
                c0, cols = _layout32[name]
                return b32t[:, c0:c0 + cols]

            hinit, cinit = c32("hinit"), c32("cinit")
            attb, av = c32("attb"), c32("av")
            b1c, b2c, b3c = c32("b1"), c32("b2"), c32("b3")

            wih1 = cload("Wih1")
            ones = cload("ones")
            ident = cload("ident")
            whh1 = cload("Whh1")
            wih2 = cload("Wih2")
            whh2 = cload("Whh2")
            wih3 = cload("Wih3")
            whh3 = cload("Whh3")
            attVT = cload("attVT")
            encT = cload("encT")
            attWT = cload("attWT")
            encg = cload("enc")
            w1T = cload("w1T")
            w2T = cload("w2T")
            w3T = cload("w3T")

            grids = {0: {"hh": whh1, "ih": None},
                     1: {"hh": whh2, "ih": wih2},
                     2: {"hh": whh3, "ih": wih3}}

            def gchunk(gr, k, m, mout=8):
                i = k * mout + m
                return gr[:, i * 128:(i + 1) * 128]

            def brow_chunk(l, m):
                c = l * 1024 + m * 128
                return brow[0:1, c:c + 128]

            # --- h ping-pong buffers; col 0 of each chunk = init.
            # buffer set 0 is read (as zeros + init col) by iteration 0.
            hbufs = [[wp.tile([128, 2 * CW], BF16, name=f"hb{l}{p}",
                              tag=f"hb{l}{p}") for l in range(3)]
                     for p in range(2)]
            for l in range(3):
                nc.gpsimd.memset(hbufs[0][l][:], 0.0)
            for p in range(2):
                for l in range(3):
                    dst = hbufs[p][l][:].rearrange("p (c u) -> p c u", c=2)
                    nc.vector.tensor_copy(
                        dst[:, :, 0:1],
                        hinit[:, 2 * l:2 * l + 2].rearrange(
                            "p (c u) -> p c u", c=2))

            # --- X.T = emb.T @ onehot -> [128, 2, TW]
            x_ps = pm.tile([128, 2 * TW], F32, name="x_ps", tag="pm")
            for m in range(2):
                nc.tensor.matmul(x_ps[:, m * TW:(m + 1) * TW],
                                 emb[:V, m * 128:(m + 1) * 128],
                                 onehot[:V, :], start=(m == 0),
                                 stop=(m == 1))
            x_sb = wp.tile([128, 2 * TW], BF16, tag="xsb")
            nc.vector.tensor_copy(x_sb[:], x_ps[:])

            # --- XW1 = W_ih1.T-grid @ X + b1sum (bias via 1-row stationary)
            # NOTE: exactly one start=True per PSUM tile lifetime — a start
            # marks the whole 2KB zero-region pending-zero, so a second start
            # would drop earlier slices' accumulation on the next write.
            xw_ps = pg.tile([128, 8 * TW], F32, name="xw_ps", tag="gates")
            for m in range(8):
                for k in range(2):
                    nc.tensor.matmul(
                        xw_ps[:, m * TW:(m + 1) * TW],
                        gchunk(wih1, k, m),
                        x_sb[:, k * TW:(k + 1) * TW],
                        start=(m == 0 and k == 0), stop=False)
                nc.tensor.matmul(xw_ps[:, m * TW:(m + 1) * TW],
                                 brow_chunk(0, m), ones[0:1, :TW],
                                 start=False, stop=(m == 7))
            xw1 = wp.tile([128, 8 * TW], BF16, tag="xw1")
            nc.vector.tensor_copy(xw1[:], xw_ps[:])

            # --- av broadcast field for the e0 fold: av_bc[h,t] = av[h]
            av_bc = wp.tile([128, 2 * TW], BF16, tag="avbc")
            for k in range(2):
                nc.vector.tensor_scalar_mul(av_bc[:, k * TW:(k + 1) * TW],
                                            ones[:, :TW], av[:, k:k + 1])

            # --- attention precompute tiles (filled mid-phase-1)
            tb = wp.tile([128, 2 * 1024], BF16, tag="tb")
            t2 = wp.tile([128, 2 * 1024], BF16, tag="t2")
            d1 = wp.tile([128, 2 * 1024], BF16, tag="d1")
            e_ps = pe_pool.tile([TW, 1024], F32, tag="eps")

            def emit_vout(m):
                # tb[:, m*1024:(m+1)*1024] = tanh(attV@encT + attb) (bf16)
                vout_ps = pm.tile([128, 512], F32, name=f"vo{m}", tag="pm")
                for h in range(2):
                    for k in range(2):
                        nc.tensor.matmul(
                            vout_ps[:],
                            gchunk(attVT, k, m, mout=2),
                            encT[:, k * 1024 + h * 512:k * 1024 + (h + 1) * 512],
                            start=(k == 0), stop=(k == 1))
                    nc.scalar.activation(
                        tb[:, m * 1024 + h * 512:m * 1024 + (h + 1) * 512],
                        vout_ps[:], AF.Tanh, bias=attb[:, m:m + 1])

            def emit_d1(q):
                sq = slice(q * 512, (q + 1) * 512)
                nc.vector.tensor_mul(t2[:, sq], tb[:, sq], tb[:, sq])
                nc.vector.tensor_scalar(d1[:, sq], t2[:, sq], -1.0, 1.0,
                                        OP.mult, OP.add)

            def emit_e0(h):
                # e_ps[t, s-half] = sum_h av[h]*tb[h,s]  (starts the e group)
                for k in range(2):
                    nc.tensor.matmul(
                        e_ps[:, h * 512:(h + 1) * 512],
                        av_bc[:, k * TW:(k + 1) * TW],
                        tb[:, k * 1024 + h * 512:k * 1024 + (h + 1) * 512],
                        start=(k == 0), stop=False)

            precompute = ([lambda m=m: emit_vout(m) for m in range(2)]
                          + [lambda q=q: emit_d1(q) for q in range(4)]
                          + [lambda h=h: emit_e0(h) for h in range(2)])

            # ---------------- Picard sweeps (wavefront order) ----------
            def emit_A(l, it):
                """hh-part of the gate PSUM accumulation (dep: 2 diagonals back)"""
                ps = pg.tile([128, 8 * TW], F32, name=f"ps{l}_{it}",
                             tag="gates")
                rb = hbufs[it % 2][l]
                for m in range(8):
                    for k in range(2):
                        nc.tensor.matmul(
                            ps[:, m * TW:(m + 1) * TW],
                            gchunk(grids[l]["hh"], k, m),
                            rb[:, k * CW:k * CW + TW],
                            start=(m == 0 and k == 0), stop=False)
                return ps

            def emit_chain(l, it, gates_sb=None, ps=None):
                """elementwise tail: sigmoid/tanh/scan -> h write"""
                wb = hbufs[(it + 1) % 2][l]
                if gates_sb is not None:
                    src6, src2 = gates_sb[:, :6 * TW], gates_sb[:, 6 * TW:]
                else:
                    src6, src2 = ps[:, :6 * TW], ps[:, 6 * TW:]
                sig = ewp.tile([128, 6 * TW], BF16, name="sig", tag="sig")
                tg = ewp.tile([128, 2 * TW], BF16, name="tg", tag="tg")
                nc.scalar.activation(sig[:], src6, AF.Sigmoid)
                nc.scalar.activation(tg[:], src2, AF.Tanh)
                z = ewp.tile([128, 2 * TW], BF16, name="z", tag="z")
                nc.vector.tensor_mul(z[:], sig[:, :2 * TW], tg[:])
                cs = ewp.tile([128, 2 * TW], BF16, name="cs", tag="cs")
                for j in range(2):
                    nc.vector.tensor_tensor_scan(
                        cs[:, j * TW:(j + 1) * TW],
                        sig[:, 2 * TW + j * TW:2 * TW + (j + 1) * TW],
                        z[:, j * TW:(j + 1) * TW],
                        cinit[:, 2 * l + j:2 * l + j + 1],
                        OP.mult, OP.add)
                tcs = ewp.tile([128, 2 * TW], BF16, name="tcs", tag="tcs")
                nc.scalar.activation(tcs[:], cs[:], AF.Tanh)
                dst = wb[:].rearrange("p (c u) -> p c u", c=2)[:, :, 1:CW]
                nc.vector.tensor_mul(
                    dst,
                    sig[:, 4 * TW:6 * TW].rearrange("p (c u) -> p c u", c=2),
                    tcs[:].rearrange("p (c u) -> p c u", c=2))

            def emit_B(l, it, ps, ih_src_override=None):
                """ih-part (+ bias / xw1 fold) closing the PSUM group, then chain"""
                if l == 0:
                    for m in range(8):
                        nc.tensor.matmul(ps[:, m * TW:(m + 1) * TW],
                                         ident[:],
                                         xw1[:, m * TW:(m + 1) * TW],
                                         start=False, stop=(m == 7))
                else:
                    src = (hbufs[(it + 1) % 2][l - 1] if ih_src_override is None
                           else ih_src_override)
                    for m in range(8):
                        for k in range(2):
                            nc.tensor.matmul(
                                ps[:, m * TW:(m + 1) * TW],
                                gchunk(grids[l]["ih"], k, m),
                                src[:, k * CW + 1:k * CW + 1 + TW],
                                start=False, stop=False)
                        nc.tensor.matmul(ps[:, m * TW:(m + 1) * TW],
                                         brow_chunk(l, m), ones[0:1, :TW],
                                         start=False, stop=(m == 7))
                emit_chain(l, it, ps=ps)

            # diagonal schedule: unit (l, it) at t = 2*it + l; extra L3 at
            # t = 2*K_IT + 1 reads layer-2's final iterate.
            units_at = {}
            for it in range(K_IT):
                for l in range(3):
                    units_at.setdefault(2 * it + l, []).append((l, it))
            TX = 2 * K_IT + 1
            units_at.setdefault(TX, []).append((2, K_IT))

            pending_A = {}
            for (l, it) in units_at.get(0, []):
                pending_A[(l, it)] = emit_A(l, it)
            pre_i = 0
            for t in range(TX + 1):
                for (l, it) in units_at.get(t, []):
                    if it == K_IT:   # extra L3 polish
                        ps = pending_A.pop((l, it))
                        emit_B(l, it, ps,
                               ih_src_override=hbufs[K_IT % 2][1])
                    else:
                        ps = pending_A.pop((l, it))
                        emit_B(l, it, ps)
                # software-pipeline: hh groups for the next diagonal
                for (l, it) in units_at.get(t + 1, []):
                    pending_A[(l, it)] = emit_A(l, it)
                # sprinkle attention precompute between diagonals
                if t >= 3 and pre_i < len(precompute):
                    precompute[pre_i]()
                    pre_i += 1
            while pre_i < len(precompute):
                precompute[pre_i]()
                pre_i += 1

            h2f = hbufs[(K_IT + 1) % 2][2]
            h2c = [h2f[:, k * CW + 1:k * CW + 1 + TW] for k in range(2)]

            # ---------------- phase 2: attention + MLP ----------------
            ws_ps = pm.tile([128, 2, TW], F32, name="ws", tag="pm")
            for m in range(2):
                for k in range(2):
                    nc.tensor.matmul(ws_ps[:, m, :],
                                     gchunk(attWT, k, m, mout=2), h2c[k],
                                     start=(m == 0 and k == 0),
                                     stop=(m == 1 and k == 1))
            u1 = wp.tile([128, 2 * TW], BF16, tag="u1")
            for m in range(2):
                nc.vector.tensor_scalar_mul(u1[:, m * TW:(m + 1) * TW],
                                            ws_ps[:, m, :], av[:, m:m + 1])
            for h in range(2):
                for k in range(2):
                    nc.tensor.matmul(
                        e_ps[:, h * 512:(h + 1) * 512],
                        u1[:, k * TW:(k + 1) * TW],
                        d1[:, k * 1024 + h * 512:k * 1024 + (h + 1) * 512],
                        start=False, stop=(k == 1))

            # softmax over s (|e| < 0.2, no max-subtraction needed)
            alpha = wp.tile([TW, 1024], BF16, tag="alpha")
            asum = wp.tile([TW, 1], F32, tag="asum")
            nc.scalar.activation(alpha[:], e_ps[:], AF.Exp, accum_out=asum[:])
            rsum = wp.tile([TW, 1], F32, tag="rsum")
            nc.vector.reciprocal(rsum[:], asum[:])
            alphan = wp.tile([TW, 1024], BF16, tag="alphan")
            nc.vector.tensor_scalar_mul(alphan[:], alpha[:], rsum[:])

            # transpose alpha -> [1024(s), TW], then ctx.T = enc.T @ a.T
            at_ps = pm.tile([128, 8 * TW], BF16, tag="pm")
            for j in range(8):
                nc.tensor.transpose(at_ps[:, j * TW:(j + 1) * TW],
                                    alphan[:, j * 128:(j + 1) * 128],
                                    ident[0:TW, 0:TW])
            at_sb = wp.tile([128, 8 * TW], BF16, tag="atsb")
            nc.vector.tensor_copy(at_sb[:], at_ps[:])
            ctx_ps = pm.tile([128, 2, TW], F32, tag="pm")
            for m in range(2):
                for k in range(8):
                    nc.tensor.matmul(ctx_ps[:, m, :],
                                     gchunk(encg, k, m, mout=2),
                                     at_sb[:, k * TW:(k + 1) * TW],
                                     start=(m == 0 and k == 0),
                                     stop=(m == 1 and k == 7))
            ctx16 = wp.tile([128, 2 * TW], BF16, tag="ctx16")
            nc.vector.tensor_copy(ctx16[:],
                                  ctx_ps[:].rearrange("p c u -> p (c u)"))

            # MLP: v = [h2; ctx]
            v1_ps = pm.tile([128, 2, TW], F32, tag="pm")
            for m in range(2):
                for k in range(4):
                    rhs = (h2c[k] if k < 2
                           else ctx16[:, (k - 2) * TW:(k - 1) * TW])
                    nc.tensor.matmul(v1_ps[:, m, :], gchunk(w1T, k, m, mout=2),
                                     rhs, start=(m == 0 and k == 0),
                                     stop=(m == 1 and k == 3))
            v1 = wp.tile([128, 2 * TW], BF16, tag="v1")
            for m in range(2):
                nc.scalar.activation(v1[:, m * TW:(m + 1) * TW], v1_ps[:, m, :],
                                     AF.Relu, bias=b1c[:, m:m + 1])
            v2_ps = pm.tile([128, 2, TW], F32, tag="pm")
            for m in range(2):
                for k in range(2):
                    nc.tensor.matmul(v2_ps[:, m, :], gchunk(w2T, k, m, mout=2),
                                     v1[:, k * TW:(k + 1) * TW],
                                     start=(m == 0 and k == 0),
                                     stop=(m == 1 and k == 1))
            v2 = wp.tile([128, 2 * TW], BF16, tag="v2")
            for m in range(2):
                nc.scalar.activation(v2[:, m * TW:(m + 1) * TW], v2_ps[:, m, :],
                                     AF.Relu, bias=b2c[:, m:m + 1])
            o_ps = pm.tile([V, TW], F32, tag="pm")
            for k in range(2):
                nc.tensor.matmul(o_ps[:], w3T[:, k * V:(k + 1) * V],
                                 v2[:, k * TW:(k + 1) * TW],
                                 start=(k == 0), stop=(k == 1))
            o_sb = wp.tile([V, TW], F32, tag="osb")
            nc.scalar.activation(o_sb[:], o_ps[:], AF.Identity,
                                 bias=b3c[:V, 0:1])
            nc.sync.dma_start(out_d[:], o_sb[:])

    nc.compile()
    return nc


def _run(inp, trace=False):
    if _NC_CACHE[0] is None:
        _NC_CACHE[0] = _build()
    nc = _NC_CACHE[0]
    from concourse.bass_utils import run_bass_kernel_spmd
    in_maps = []
    for k in range(NCORES):
        b16, row, b32 = _pack(inp, k)
        in_maps.append({"blob16": b16, "brow": row, "blob32": b32})
    res = run_bass_kernel_spmd(nc, in_maps, list(range(NCORES)), trace=trace)
    out = np.zeros((TN, 1, V), np.float32)
    for k in range(NCORES):
        o = res.results[k]["out"]          # [47, TW]
        c0 = 0 if k == 0 else TW - CHUNK
        out[CHUNK * k:CHUNK * k + CHUNK, 0, :] = o[:, c0:c0 + CHUNK].T
    return out, res


def kernel(**inputs) -> np.ndarray:
    inp = {k: np.asarray(v) if not np.isscalar(v) else v
           for k, v in inputs.items()}
    out, _ = _run(inp, trace=False)
    return out


# revision 14
# speedup vs baseline: 2.1976x; 1.2916x over previous
"""Trainium2 Bass kernel for a 3-layer LSTM decoder with Bahdanau attention.

Strategy (8 NeuronCores, data-parallel over time windows):
  Each core processes a 48-step time window (32-step output chunk + 16-step
  halo) and solves the teacher-forced recurrence by Picard fixed-point
  iteration (Gauss-Seidel over layers): K bf16 sweeps + one extra layer-3
  polish, with the linear cell recurrence solved exactly per sweep by the
  hardware scan. Weights are tiny (sigma=0.05) so the map is strongly
  contractive.

  Attention uses a 1st-order Taylor expansion of tanh(VOut + att_W h2 + b)
  around the t-independent base; the loop-invariant term e0 = av @ tanh(base)
  is folded into the e-matmul via a broadcast-av stationary, and av itself is
  folded into att_W's rows on the host.

  Everything is bf16 on the PE (1 cycle/row); PSUM accumulation is fp32.
  The three activation-table loads (sigmoid/tanh/exp) are pulled to t=0 by
  dummy activations so they hide under the weight DMA, constants stream in a
  few large group DMAs ordered by first use, and layer-3 elementwise chains
  (off the critical path) are emitted one diagonal late so the Act/DVE
  queues stay clear for the layer-2 recurrence.
"""

import numpy as np

H = 256
V = 47
S = 1024
TN = 256
TW = 48          # per-core window (32 out + 16 halo)
CW = TW + 1      # window + init column
CHUNK = 32
NCORES = 8
K_IT = 4         # bf16 Picard sweeps (gauss-seidel over layers)
# one extra layer-3 polish unit after the K sweeps (always on)

# ------------------------------------------------------- blob layout (groups)
_layout16 = {}      # name -> (group, col offset within group, cols)
_groups = []        # [(gname, total cols)]


def _group16(gname, *tensors):
    c = 0
    for name, cols in tensors:
        _layout16[name] = (gname, c, cols)
        c += cols
    _groups.append((gname, c))


_group16("g0", ("onehot", TW), ("emb", 256), ("ones", TW), ("ident", 128))
_group16("gWih1", ("Wih1", 2048))
_group16("gWhh1", ("Whh1", 2048))
_group16("g3", ("Wih2", 2048), ("Whh2", 2048))
_group16("g4", ("Wih3", 2048), ("Whh3", 2048))
_group16("g5", ("attVT", 512), ("encT", 2048))
_group16("g6", ("attWT", 512), ("enc", 2048), ("w1T", 1024), ("w2T", 512),
         ("w3T", 2 * V))
_GCOLS = dict(_groups)
BLOB16_C = sum(c for _, c in _groups)
_GOFF = {}
_off = 0
for _g, _c in _groups:
    _GOFF[_g] = _off
    _off += _c

# bias row: 3 LSTM layers' combined gate biases, then mlp b1, b2, b3
_ROWL = {"lstm": (0, 3 * 1024), "b1": (3 * 1024, 256),
         "b2": (3 * 1024 + 256, 256), "b3": (3 * 1024 + 512, V)}
ROW_C = 3 * 1024 + 512 + V

_layout32 = {}
_c32 = 0


def _span32(name, cols):
    global _c32
    _layout32[name] = (_c32, cols)
    _c32 += cols


_span32("hinit", 6)
_span32("cinit", 6)
_span32("attb", 2)
_span32("av", 2)
BLOB32_C = _c32


def _gate_perm():
    # reorder gates i,f,g,o -> i,f,o,g so sigmoid gates are contiguous
    r = np.arange(H)
    return np.concatenate([r, H + r, 3 * H + r, 2 * H + r])


def _grid_wT(W):
    """W [out,in] -> W.T chunk grid [128, (in//128)*(out//128)*128]."""
    WT = np.ascontiguousarray(W.T.astype(np.float32))
    kin, mout = WT.shape[0] // 128, WT.shape[1] // 128
    g = np.empty((128, kin * mout * 128), np.float32)
    for k in range(kin):
        for m in range(mout):
            g[:, (k * mout + m) * 128:(k * mout + m + 1) * 128] = \
                WT[k * 128:(k + 1) * 128, m * 128:(m + 1) * 128]
    return g


def _hmaj(v):
    n = v.shape[0] // 128
    return np.ascontiguousarray(v.reshape(n, 128).T.astype(np.float32))


def _pack(inp, core):
    import ml_dtypes
    BF = ml_dtypes.bfloat16
    b16 = np.zeros((128, BLOB16_C), BF)
    row = np.zeros((1, ROW_C), BF)
    b32 = np.zeros((128, BLOB32_C), np.float32)
    perm = _gate_perm()

    def put16(name, arr):
        g, c0, cols = _layout16[name]
        c0 += _GOFF[g]
        b16[:arr.shape[0], c0:c0 + arr.shape[1]] = arr.astype(BF)

    def put32(name, arr):
        c0, cols = _layout32[name]
        b32[:arr.shape[0], c0:c0 + arr.shape[1]] = arr

    lo = 0 if core == 0 else CHUNK * core - (TW - CHUNK)
    Y = np.asarray(inp["Y"]).astype(np.int64)[lo:lo + TW]
    oh = np.zeros((V, TW), np.float32)
    oh[Y, np.arange(TW)] = 1.0
    put16("onehot", oh)
    put16("emb", np.asarray(inp["emb"]).astype(np.float32))
    put16("ones", np.ones((128, TW), np.float32))
    put16("ident", np.eye(128, dtype=np.float32))
    for l in (1, 2, 3):
        put16(f"Wih{l}", _grid_wT(np.asarray(inp[f"W_ih{l}"])[perm]))
        put16(f"Whh{l}", _grid_wT(np.asarray(inp[f"W_hh{l}"])[perm]))
        bsum = (np.asarray(inp[f"b_ih{l}"]) + np.asarray(inp[f"b_hh{l}"]))[perm]
        row[0, (l - 1) * 1024:l * 1024] = bsum.astype(np.float32)
    row[0, _ROWL["b1"][0]:_ROWL["b1"][0] + 256] = \
        np.asarray(inp["mlp_b1"]).astype(np.float32)
    row[0, _ROWL["b2"][0]:_ROWL["b2"][0] + 256] = \
        np.asarray(inp["mlp_b2"]).astype(np.float32)
    row[0, _ROWL["b3"][0]:_ROWL["b3"][0] + V] = \
        np.asarray(inp["mlp_b3"]).astype(np.float32)
    put16("attVT", _grid_wT(np.asarray(inp["att_V"])))
    # av folded into att_W rows: u1 = (av ⊙ att_W) @ h2 directly
    attWs = (np.asarray(inp["att_W"]).astype(np.float32)
             * np.asarray(inp["att_vector"]).astype(np.float32)[0][:, None])
    put16("attWT", _grid_wT(attWs))
    enc = np.asarray(inp["outEncoder"]).astype(np.float32)
    encT = np.ascontiguousarray(enc.T)
    eT = np.empty((128, 2048), np.float32)
    for k in range(2):
        eT[:, k * 1024:(k + 1) * 1024] = encT[k * 128:(k + 1) * 128, :]
    put16("encT", eT)
    eg = np.empty((128, 16 * 128), np.float32)
    for k in range(8):
        for m in range(2):
            eg[:, (k * 2 + m) * 128:(k * 2 + m + 1) * 128] = \
                enc[k * 128:(k + 1) * 128, m * 128:(m + 1) * 128]
    put16("enc", eg)
    put16("w1T", _grid_wT(np.asarray(inp["mlp_w1"])))
    put16("w2T", _grid_wT(np.asarray(inp["mlp_w2"])))
    w3T = np.ascontiguousarray(np.asarray(inp["mlp_w3"]).astype(np.float32).T)
    g3 = np.empty((128, 2 * V), np.float32)
    for k in range(2):
        g3[:, k * V:(k + 1) * V] = w3T[k * 128:(k + 1) * 128, :]
    put16("w3T", g3)

    if core == 0:
        hi = np.concatenate([_hmaj(np.asarray(inp["h"])[l, 0]) for l in range(3)], 1)
        ci = np.concatenate([_hmaj(np.asarray(inp["c"])[l, 0]) for l in range(3)], 1)
        put32("hinit", hi)
        put32("cinit", ci)
    put32("attb", _hmaj(np.asarray(inp["att_b"])[:, 0]))
    put32("av", _hmaj(np.asarray(inp["att_vector"])[0]))
    return b16, row, b32


# ------------------------------------------------------------------- builder
_NC_CACHE = [None]


def _build():
    import concourse.bacc as bacc
    import concourse.mybir as mybir
    from concourse import tile

    F32 = mybir.dt.float32
    BF16 = mybir.dt.bfloat16
    AF = mybir.ActivationFunctionType
    OP = mybir.AluOpType

    nc = bacc.Bacc("TRN2", target_bir_lowering=False, debug=False,
                   num_devices=NCORES)
    b16_d = nc.dram_tensor("blob16", [128, BLOB16_C], BF16,
                           kind="ExternalInput").ap()
    row_d = nc.dram_tensor("brow", [1, ROW_C], BF16, kind="ExternalInput").ap()
    b32_d = nc.dram_tensor("blob32", [128, BLOB32_C], F32,
                           kind="ExternalInput").ap()
    out_d = nc.dram_tensor("out", [V, TW], F32, kind="ExternalOutput").ap()

    with tile.TileContext(nc) as tc:
        import contextlib
        ctx = contextlib.ExitStack()
        with ctx:
            cp = ctx.enter_context(tc.tile_pool(name="consts", bufs=1))
            wp = ctx.enter_context(tc.tile_pool(name="work", bufs=1))
            ewp = ctx.enter_context(tc.tile_pool(name="ew", bufs=4))
            pg = ctx.enter_context(tc.tile_pool(name="pgates", bufs=5,
                                                space="PSUM"))
            pe_pool = ctx.enter_context(tc.tile_pool(name="peps", bufs=1,
                                                     space="PSUM"))
            pm = ctx.enter_context(tc.tile_pool(name="pmisc", bufs=1,
                                                space="PSUM"))

            # --- act-table warmup: pull the 3 table loads to t=0
            warm = wp.tile([1, 4], F32, tag="warm")
            nc.gpsimd.memset(warm[:], 0.0)
            nc.scalar.activation(warm[:, 1:2], warm[:, 0:1], AF.Sigmoid)
            nc.scalar.activation(warm[:, 2:3], warm[:, 0:1], AF.Tanh)
            nc.scalar.activation(warm[:, 3:4], warm[:, 0:1], AF.Exp)

            # --- grouped constant loads (one DMA per group, first-use order)
            gtiles = {}

            def gload(gname):
                cols = _GCOLS[gname]
                t = cp.tile([128, cols], BF16, name=gname, tag=gname)
                o = _GOFF[gname]
                nc.sync.dma_start(t[:], b16_d[:, o:o + cols])
                gtiles[gname] = t
                return t

            def ct(name):
                g, c0, cols = _layout16[name]
                return gtiles[g][:, c0:c0 + cols]

            gload("g0")
            gload("gWih1")
            brow = cp.tile([1, ROW_C], BF16, name="brow", tag="brow")
            nc.sync.dma_start(brow[:], row_d[:])
            b32t = cp.tile([128, BLOB32_C], F32, name="b32", tag="b32")
            nc.sync.dma_start(b32t[:], b32_d[:])
            gload("gWhh1")
            gload("g3")
            gload("g4")
            gload("g5")
            gload("g6")

            def c32(name):
                c0, cols = _layout32[name]
                return b32t[:, c0:c0 + cols]

            hinit, cinit = c32("hinit"), c32("cinit")
            attb, av = c32("attb"), c32("av")
            onehot, emb = ct("onehot"), ct("emb")
            ones, ident = ct("ones"), ct("ident")
            attVT, encT, attWT = ct("attVT"), ct("encT"), ct("attWT")
            encg, w1T, w2T, w3T = ct("enc"), ct("w1T"), ct("w2T"), ct("w3T")

            grids = {0: {"hh": ct("Whh1"), "ih": None},
                     1: {"hh": ct("Whh2"), "ih": ct("Wih2")},
                     2: {"hh": ct("Whh3"), "ih": ct("Wih3")}}
            wih1 = ct("Wih1")

            def gchunk(gr, k, m, mout=8):
                i = k * mout + m
                return gr[:, i * 128:(i + 1) * 128]

            def brow_lstm(l, m):
                c = l * 1024 + m * 128
                return brow[0:1, c:c + 128]

            def brow_mlp(name, m, w=128):
                c = _ROWL[name][0] + m * w
                return brow[0:1, c:c + w]

            # --- h ping-pong buffers; col 0 of each chunk = init.
            # buffer set 0 is read (as zeros + init col) by iteration 0.
            hbufs = [[wp.tile([128, 2 * CW], BF16, name=f"hb{l}{p}",
                              tag=f"hb{l}{p}") for l in range(3)]
                     for p in range(2)]
            for l in range(3):
                nc.gpsimd.memset(hbufs[0][l][:], 0.0)
            for p in range(2):
                for l in range(3):
                    dst = hbufs[p][l][:].rearrange("p (c u) -> p c u", c=2)
                    nc.vector.tensor_copy(
                        dst[:, :, 0:1],
                        hinit[:, 2 * l:2 * l + 2].rearrange(
                            "p (c u) -> p c u", c=2))

            # --- X.T = emb.T @ onehot -> [128, 2, TW]
            x_ps = pm.tile([128, 2 * TW], F32, name="x_ps", tag="pm")
            for m in range(2):
                nc.tensor.matmul(x_ps[:, m * TW:(m + 1) * TW],
                                 emb[:V, m * 128:(m + 1) * 128],
                                 onehot[:V, :], start=(m == 0),
                                 stop=(m == 1))
            x_sb = wp.tile([128, 2 * TW], BF16, tag="xsb")
            nc.vector.tensor_copy(x_sb[:], x_ps[:])

            # --- XW1 = W_ih1.T-grid @ X + b1sum (bias via 1-row stationary)
            # NOTE: exactly one start=True per PSUM tile lifetime — a start
            # marks the whole 2KB zero-region pending-zero, so a second start
            # would drop earlier slices' accumulation on the next write.
            xw_ps = pg.tile([128, 8 * TW], F32, name="xw_ps", tag="gates")
            for m in range(8):
                for k in range(2):
                    nc.tensor.matmul(
                        xw_ps[:, m * TW:(m + 1) * TW],
                        gchunk(wih1, k, m),
                        x_sb[:, k * TW:(k + 1) * TW],
                        start=(m == 0 and k == 0), stop=False)
                nc.tensor.matmul(xw_ps[:, m * TW:(m + 1) * TW],
                                 brow_lstm(0, m), ones[0:1, :TW],
                                 start=False, stop=(m == 7))
            xw1 = wp.tile([128, 8 * TW], BF16, tag="xw1")
            nc.vector.tensor_copy(xw1[:], xw_ps[:])

            # --- av broadcast field for the e0 fold: av_bc[h,t] = av[h]
            av_bc = wp.tile([128, 2 * TW], BF16, tag="avbc")
            for k in range(2):
                nc.vector.tensor_scalar_mul(av_bc[:, k * TW:(k + 1) * TW],
                                            ones[:, :TW], av[:, k:k + 1])

            # --- attention precompute tiles (filled mid-phase-1)
            tb = wp.tile([128, 2 * 1024], BF16, tag="tb")
            t2 = wp.tile([128, 2 * 1024], BF16, tag="t2")
            d1 = wp.tile([128, 2 * 1024], BF16, tag="d1")
            e_ps = pe_pool.tile([TW, 1024], F32, tag="eps")

            def emit_vout(m, h):
                # tb[:, m-chunk, h-half] = tanh(attV@encT + attb) (bf16)
                vout_ps = pm.tile([128, 512], F32, name=f"vo{m}{h}", tag="pm")
                for k in range(2):
                    nc.tensor.matmul(
                        vout_ps[:],
                        gchunk(attVT, k, m, mout=2),
                        encT[:, k * 1024 + h * 512:k * 1024 + (h + 1) * 512],
                        start=(k == 0), stop=(k == 1))
                nc.scalar.activation(
                    tb[:, m * 1024 + h * 512:m * 1024 + (h + 1) * 512],
                    vout_ps[:], AF.Tanh, bias=attb[:, m:m + 1])

            def emit_d1(q):
                sq = slice(q * 512, (q + 1) * 512)
                nc.vector.tensor_mul(t2[:, sq], tb[:, sq], tb[:, sq])
                nc.vector.tensor_scalar(d1[:, sq], t2[:, sq], -1.0, 1.0,
                                        OP.mult, OP.add)

            def emit_e0(h):
                # e_ps[t, s-half] = sum_h av[h]*tb[h,s]  (starts the e group)
                for k in range(2):
                    nc.tensor.matmul(
                        e_ps[:, h * 512:(h + 1) * 512],
                        av_bc[:, k * TW:(k + 1) * TW],
                        tb[:, k * 1024 + h * 512:k * 1024 + (h + 1) * 512],
                        start=(k == 0), stop=False)

            precompute = ([lambda m=m, h=h: emit_vout(m, h)
                           for m in range(2) for h in range(2)]
                          + [lambda q=q: emit_d1(q) for q in range(4)]
                          + [lambda h=h: emit_e0(h) for h in range(2)])

            # ---------------- Picard sweeps (wavefront order) ----------
            def emit_A(l, it):
                """hh-part of the gate PSUM accumulation (dep: 2 diagonals back)"""
                ps = pg.tile([128, 8 * TW], F32, name=f"ps{l}_{it}",
                             tag="gates")
                rb = hbufs[it % 2][l]
                for m in range(8):
                    for k in range(2):
                        nc.tensor.matmul(
                            ps[:, m * TW:(m + 1) * TW],
                            gchunk(grids[l]["hh"], k, m),
                            rb[:, k * CW:k * CW + TW],
                            start=(m == 0 and k == 0), stop=False)
                return ps

            def emit_B(l, it, ps, ih_src_override=None):
                """ih-part (+ bias / xw1 fold) closing the PSUM group"""
                if l == 0:
                    for m in range(8):
                        nc.tensor.matmul(ps[:, m * TW:(m + 1) * TW],
                                         ident[:],
                                         xw1[:, m * TW:(m + 1) * TW],
                                         start=False, stop=(m == 7))
                else:
                    src = (hbufs[(it + 1) % 2][l - 1] if ih_src_override is None
                           else ih_src_override)
                    for m in range(8):
                        for k in range(2):
                            nc.tensor.matmul(
                                ps[:, m * TW:(m + 1) * TW],
                                gchunk(grids[l]["ih"], k, m),
                                src[:, k * CW + 1:k * CW + 1 + TW],
                                start=False, stop=False)
                        nc.tensor.matmul(ps[:, m * TW:(m + 1) * TW],
                                         brow_lstm(l, m), ones[0:1, :TW],
                                         start=False, stop=(m == 7))

            def emit_chain(l, it, ps):
                """elementwise tail: sigmoid/tanh/scan -> h write"""
                wb = hbufs[(it + 1) % 2][l]
                sig = ewp.tile([128, 6 * TW], BF16, name="sig", tag="sig")
                tg = ewp.tile([128, 2 * TW], BF16, name="tg", tag="tg")
                nc.scalar.activation(sig[:], ps[:, :6 * TW], AF.Sigmoid)
                nc.scalar.activation(tg[:], ps[:, 6 * TW:], AF.Tanh)
                z = ewp.tile([128, 2 * TW], BF16, name="z", tag="z")
                nc.vector.tensor_mul(z[:], sig[:, :2 * TW], tg[:])
                cs = ewp.tile([128, 2 * TW], BF16, name="cs", tag="cs")
                for j in range(2):
                    nc.vector.tensor_tensor_scan(
                        cs[:, j * TW:(j + 1) * TW],
                        sig[:, 2 * TW + j * TW:2 * TW + (j + 1) * TW],
                        z[:, j * TW:(j + 1) * TW],
                        cinit[:, 2 * l + j:2 * l + j + 1],
                        OP.mult, OP.add)
                tcs = ewp.tile([128, 2 * TW], BF16, name="tcs", tag="tcs")
                nc.scalar.activation(tcs[:], cs[:], AF.Tanh)
                dst = wb[:].rearrange("p (c u) -> p c u", c=2)[:, :, 1:CW]
                nc.vector.tensor_mul(
                    dst,
                    sig[:, 4 * TW:6 * TW].rearrange("p (c u) -> p c u", c=2),
                    tcs[:].rearrange("p (c u) -> p c u", c=2))

            # diagonal schedule: unit (l, it) at t = 2*it + l; extra L3 at
            # t = 2*K_IT + 1 reads layer-2's final iterate. Layer-3 chains
            # (pure sinks for 2 diagonals) are emitted one diagonal late so
            # their Act/DVE ops queue behind the critical layer-1/2 chains.
            units_at = {}
            for it in range(K_IT):
                for l in range(3):
                    units_at.setdefault(2 * it + l, []).append((l, it))
            TX = 2 * K_IT + 1
            units_at.setdefault(TX, []).append((2, K_IT))

            # NOTE on ordering: the Tile framework derives dependencies from
            # program order, so a reader emitted before its writer would bind
            # to stale data. l=2 A-groups therefore emit at the END of each
            # step, after the (possibly deferred) l=2 chain they read.
            pending_A = {}
            for u in units_at.get(0, []):
                pending_A[u] = emit_A(*u)
            pre_i = 0
            deferred = None
            for t in range(TX + 1):
                us = units_at.get(t, [])
                nxt = units_at.get(t + 1, [])
                for (l, it) in sorted(us, key=lambda u: u[0] == 2):
                    ps = pending_A[(l, it)]
                    emit_B(l, it, ps,
                           ih_src_override=(hbufs[K_IT % 2][1]
                                            if it == K_IT else None))
                for u in nxt:
                    if u[0] < 2:
                        pending_A[u] = emit_A(*u)
                # critical chains first (l=0 feeds next diagonal's l=1)
                for (l, it) in us:
                    if l < 2:
                        emit_chain(l, it, pending_A.pop((l, it)))
                if deferred is not None:
                    emit_chain(*deferred, pending_A.pop(deferred))
                    deferred = None
                l3 = [u for u in us if u[0] == 2]
                defer_now = (l3[0] if (l3 and t != TX
                                       and any(u[0] < 2 for u in us))
                             else None)
                if l3 and defer_now is None:
                    emit_chain(*l3[0], pending_A.pop(l3[0]))
                deferred = defer_now
                for u in nxt:
                    if u[0] == 2:
                        pending_A[u] = emit_A(*u)
                # sprinkle attention precompute between diagonals (2/diag);
                # needs g5 DMA'd, so start late enough
                if t >= 4:
                    for _ in range(2):
                        if pre_i < len(precompute):
                            precompute[pre_i]()
                            pre_i += 1
            while pre_i < len(precompute):
                precompute[pre_i]()
                pre_i += 1

            h2f = hbufs[(K_IT + 1) % 2][2]
            h2c = [h2f[:, k * CW + 1:k * CW + 1 + TW] for k in range(2)]

            # ---------------- phase 2: attention + MLP ----------------
            # u1 = (av ⊙ attW) @ h2 (av pre-folded on host)
            ws_ps = pm.tile([128, 2, TW], F32, name="ws", tag="pm")
            for m in range(2):
                for k in range(2):
                    nc.tensor.matmul(ws_ps[:, m, :],
                                     gchunk(attWT, k, m, mout=2), h2c[k],
                                     start=(m == 0 and k == 0),
                                     stop=(m == 1 and k == 1))
            u1 = wp.tile([128, 2 * TW], BF16, tag="u1")
            nc.vector.tensor_copy(u1[:],
                                  ws_ps[:].rearrange("p c u -> p (c u)"))
            for h in range(2):
                for k in range(2):
                    nc.tensor.matmul(
                        e_ps[:, h * 512:(h + 1) * 512],
                        u1[:, k * TW:(k + 1) * TW],
                        d1[:, k * 1024 + h * 512:k * 1024 + (h + 1) * 512],
                        start=False, stop=(k == 1))

            # softmax over s (|e| < 0.2, no max-subtraction needed)
            alpha = wp.tile([TW, 1024], BF16, tag="alpha")
            asum = wp.tile([TW, 1], F32, tag="asum")
            nc.scalar.activation(alpha[:], e_ps[:], AF.Exp, accum_out=asum[:])
            rsum = wp.tile([TW, 1], F32, tag="rsum")
            nc.vector.reciprocal(rsum[:], asum[:])
            alphan = wp.tile([TW, 1024], BF16, tag="alphan")
            nc.vector.tensor_scalar_mul(alphan[:], alpha[:], rsum[:])

            # transpose alpha -> [1024(s), TW], then ctx.T = enc.T @ a.T
            at_ps = pm.tile([128, 8 * TW], BF16, tag="pm")
            for j in range(8):
                nc.tensor.transpose(at_ps[:, j * TW:(j + 1) * TW],
                                    alphan[:, j * 128:(j + 1) * 128],
                                    ident[0:TW, 0:TW])
            at_sb = wp.tile([128, 8 * TW], BF16, tag="atsb")
            nc.vector.tensor_copy(at_sb[:], at_ps[:])
            ctx_ps = pm.tile([128, 2, TW], F32, tag="pm")
            for m in range(2):
                for k in range(8):
                    nc.tensor.matmul(ctx_ps[:, m, :],
                                     gchunk(encg, k, m, mout=2),
                                     at_sb[:, k * TW:(k + 1) * TW],
                                     start=(m == 0 and k == 0),
                                     stop=(m == 1 and k == 7))
            ctx16 = wp.tile([128, 2 * TW], BF16, tag="ctx16")
            nc.vector.tensor_copy(ctx16[:],
                                  ctx_ps[:].rearrange("p c u -> p (c u)"))

            # MLP: v = [h2; ctx]; biases via 1-row stationaries, relu on DVE
            v1_ps = pm.tile([128, 2, TW], F32, tag="pm")
            for m in range(2):
                for k in range(4):
                    rhs = (h2c[k] if k < 2
                           else ctx16[:, (k - 2) * TW:(k - 1) * TW])
                    nc.tensor.matmul(v1_ps[:, m, :], gchunk(w1T, k, m, mout=2),
                                     rhs, start=(m == 0 and k == 0),
                                     stop=False)
                nc.tensor.matmul(v1_ps[:, m, :], brow_mlp("b1", m),
                                 ones[0:1, :TW], start=False, stop=(m == 1))
            v1 = wp.tile([128, 2 * TW], BF16, tag="v1")
            nc.vector.tensor_scalar_max(
                v1[:], v1_ps[:].rearrange("p c u -> p (c u)"), 0.0)
            v2_ps = pm.tile([128, 2, TW], F32, tag="pm")
            for m in range(2):
                for k in range(2):
                    nc.tensor.matmul(v2_ps[:, m, :], gchunk(w2T, k, m, mout=2),
                                     v1[:, k * TW:(k + 1) * TW],
                                     start=(m == 0 and k == 0), stop=False)
                nc.tensor.matmul(v2_ps[:, m, :], brow_mlp("b2", m),
                                 ones[0:1, :TW], start=False, stop=(m == 1))
            v2 = wp.tile([128, 2 * TW], BF16, tag="v2")
            nc.vector.tensor_scalar_max(
                v2[:], v2_ps[:].rearrange("p c u -> p (c u)"), 0.0)
            o_ps = pm.tile([V, TW], F32, tag="pm")
            for k in range(2):
                nc.tensor.matmul(o_ps[:], w3T[:, k * V:(k + 1) * V],
                                 v2[:, k * TW:(k + 1) * TW],
                                 start=(k == 0), stop=False)
            nc.tensor.matmul(o_ps[:], brow_mlp("b3", 0, V), ones[0:1, :TW],
                             start=False, stop=True)
            o_sb = wp.tile([V, TW], F32, tag="osb")
            nc.vector.tensor_copy(o_sb[:], o_ps[:])
            nc.sync.dma_start(out_d[:], o_sb[:])

    nc.compile()
    return nc


def _run(inp, trace=False):
    if _NC_CACHE[0] is None:
        _NC_CACHE[0] = _build()
    nc = _NC_CACHE[0]
    from concourse.bass_utils import run_bass_kernel_spmd
    in_maps = []
    for k in range(NCORES):
        b16, row, b32 = _pack(inp, k)
        in_maps.append({"blob16": b16, "brow": row, "blob32": b32})
    res = run_bass_kernel_spmd(nc, in_maps, list(range(NCORES)), trace=trace)
    out = np.zeros((TN, 1, V), np.float32)
    for k in range(NCORES):
        o = res.results[k]["out"]          # [47, TW]
        c0 = 0 if k == 0 else TW - CHUNK
        out[CHUNK * k:CHUNK * k + CHUNK, 0, :] = o[:, c0:c0 + CHUNK].T
    return out, res


def kernel(**inputs) -> np.ndarray:
    inp = {k: np.asarray(v) if not np.isscalar(v) else v
           for k, v in inputs.items()}
    out, _ = _run(inp, trace=False)
    return out
